# revision 29
# baseline (speedup 1.0000x reference)
"""Trainium2 Bass kernel for 16-head MHA (B=2, T=2048, E=1024), SPMD on 8 cores.

Sharding: data-parallel over batch (2) x tensor-parallel over heads (4 groups
of 4 heads). Each core computes, for its (batch b, head-group g):
  qk^T projection (feature-major), v projection (token-major),
  shifted-softmax attention via an augmented-row matmul trick, and a partial
  output projection over its 256 embedding columns. The host sums the 4
  partial projections per batch.

Softmax shift: an approximate per-query max over all keys is computed
on-device from an fp8e4m3 copy of q/k using DoubleRow-mode matmuls (K=256
contraction at 0.5 cycles/row -> half the PE cost of the fp16 equivalent) in
[i, j] orientation + wide free-dim DVE reduce_max(negate=True). The estimate
is within +-15 of the true max (measured on N(0,1) data), so exp outputs are
bounded by e^15: pT is bf16 (range e^+-88) and the fp32 PSUM accumulations
are safe. The shift is folded into the main QK^T matmul as a rank-1
augmented row, so scores arrive in PSUM already shifted:
S'[j,i] = 8*q_i.k_j - M_i. exp() runs on ACT straight out of PSUM over
[128,1024] two-bank tiles. The softmax denominator comes for free from a
ones-column appended to V.
"""

import sys

sys.path.insert(0, "/opt/trn_rl_repo")

import numpy as np

import concourse.bass as bass
import concourse.mybir as mybir
import concourse.tile as tile_mod
from concourse.masks import make_identity

F32 = mybir.dt.float32
F16 = mybir.dt.float16
BF16 = mybir.dt.bfloat16
F8 = mybir.dt.float8e4

B, T, E = 2, 2048, 1024
H_TOTAL, D = 16, 64
N_CORES = 8
GROUPS = 4          # head-group (tensor) parallelism
HPG = H_TOTAL // GROUPS  # 4 heads per group
DV = HPG * D        # 256: v width / out-proj contraction per core
FQK = 2 * DV        # 512: q+k feature rows per core
SCALE = float(np.sqrt(D))  # reference MULTIPLIES scores by sqrt(d)

NE = E // 128       # 8 e-chunks
NT_TILE = T // 128  # 16 token tiles
NT_CHUNK = T // 512  # 4 token chunks


# ---------------------------------------------------------------------------
# Workaround: this walrus build only accepts ONE sem wait per instruction.
# After Tile scheduling, split every multi-wait instruction: the overflow
# waits move onto same-engine NoOps inserted immediately before it.
def _split_multi_waits(nc):
    for f in nc.m.functions:
        for bb in f.blocks:
            out = []
            for inst in bb.instructions:
                si = getattr(inst, "sync_info", None)
                if si is not None and si.on_wait and len(si.on_wait) > 1:
                    extras = list(si.on_wait[:-1])
                    si.on_wait = list(si.on_wait[-1:])
                    for w in extras:
                        nop = mybir.InstNoOp(
                            name=f"I-{nc.next_id()}", ins=[], outs=[]
                        )
                        nop.engine = inst.engine
                        nop.sync_info = mybir.SyncInfo(on_wait=[w], on_update=[])
                        out.append(nop)
                out.append(inst)
            bb.instructions[:] = out


# ---------------------------------------------------------------------------
# Device program (identical on every core; per-core data differs)
def _emit_body(nc, tc, dram, ctx_pools, dbg=None):
    xT_d, wqkT_d, wvT_d, woutT_d, y_d = dram
    persist = ctx_pools["persist"]

    # persistent SBUF
    qk_sb = [persist.tile([128, T], F16, tag=f"qk{i}", name=f"qk{i}") for i in range(FQK // 128)]
    # v as one [128, 16*256] tile: [t-tile partition, (jt, dv)] layout
    v_sb = persist.tile([128, NT_TILE * DV], F16, tag="v", name="v")
    oall_sb = [persist.tile([128, T], F16, tag=f"oall{i}", name=f"oall{i}") for i in range(DV // 128)]
    wout_sb = [persist.tile([128, E], F16, tag=f"wout{i}", name=f"wout{i}") for i in range(DV // 128)]
    identity = persist.tile([128, 128], F32, tag="identity", name="identity")
    make_identity(nc, identity)
    for i in range(DV // 128):
        nc.sync.dma_start(out=wout_sb[i], in_=woutT_d[i * 128:(i + 1) * 128, :])

    with (
        tc.tile_pool(name="aug", bufs=1) as augp,
        tc.tile_pool(name="vaug", bufs=1) as vaugp,
        tc.tile_pool(name="mx", bufs=1) as mxsb,
        tc.tile_pool(name="pmx", bufs=3, space="PSUM") as pxp,
    ):
        k_augs, q_augs, vaugs, m2s = [], [], [], []

        def build_qk_aug(h):
            # [128, T]: rows 0:64 = q|k, row 64 = -M slot (q, zero-init) /
            # ones (k), rows 65:128 = zero K-padding. Odd heads' rows live
            # at partitions 64..127 of qk_sb -> partition-shifting DMA.
            odd = h % 2 == 1
            q_tile, k_tile = h // 2, 2 + h // 2
            k_aug = augp.tile([128, T], F16, tag=f"kaug{h}", name=f"kaug{h}")
            q_aug = augp.tile([128, T], F16, tag=f"qaug{h}", name=f"qaug{h}")
            lo = D if odd else 0
            for cc in range(NT_CHUNK):
                cs = slice(cc * 512, (cc + 1) * 512)
                nc.sync.dma_start(out=k_aug[0:D, cs], in_=qk_sb[k_tile][lo:lo + D, cs])
                nc.sync.dma_start(out=q_aug[0:D, cs], in_=qk_sb[q_tile][lo:lo + D, cs])
            nc.gpsimd.memset(q_aug[D:128, :], 0.0)
            nc.gpsimd.memset(k_aug[D:128, :], 0.0)
            nc.gpsimd.memset(k_aug[D:D + 1, :], 1.0)
            k_augs.append(k_aug)
            q_augs.append(q_aug)

        # ---- max-pass emitters ----------------------------------------
        # fp16 scores in [i-part, j-free] orientation (q-stationary,
        # K=128 zero-padded); row max = narrow negated free-axis reduces.
        # q_aug row D is still zero when head h's own max pass runs, so
        # the augmented row contributes nothing here.
        def max_it(h, it):
            # one query tile: 4 matmuls + 4 narrow negated reduces
            if it == 0:
                m2 = mxsb.tile([128, NT_TILE * 4], F32, tag="m2", name="m2", bufs=2)
                m2s.append(m2)
            m2r = m2s[h].rearrange("p (j i) -> p j i", i=NT_TILE)
            q_aug, k_aug = q_augs[h], k_augs[h]
            for jc in range(4):
                ps = pxp.tile([128, 512], F32, tag="mx", name="mx")
                nc.tensor.matmul(
                    ps,
                    q_aug[:, it * 128:(it + 1) * 128],
                    k_aug[:, jc * 512:(jc + 1) * 512],
                    start=True,
                    stop=True,
                )
                nc.vector.reduce_max(
                    out=m2r[:, jc, it:it + 1], in_=ps,
                    axis=mybir.AxisListType.X, negate=True,
                )

        def max_finish(h, ic):
            # combine quarters for one 4-it chunk (values are negated:
            # min == -max), transpose to a row, DMA into q_aug row D.
            q_aug = q_augs[h]
            i0 = ic * NT_CHUNK
            m2r = m2s[h].rearrange("p (j i) -> p j i", i=NT_TILE)
            t1 = mxsb.tile([128, 4], F32, tag="t1", name="t1", bufs=2)
            t2 = mxsb.tile([128, 4], F32, tag="t2", name="t2", bufs=2)
            nc.vector.tensor_tensor(out=t1, in0=m2r[:, 0, i0:i0 + 4],
                                    in1=m2r[:, 1, i0:i0 + 4],
                                    op=mybir.AluOpType.min)
            nc.vector.tensor_tensor(out=t2, in0=m2r[:, 2, i0:i0 + 4],
                                    in1=m2r[:, 3, i0:i0 + 4],
                                    op=mybir.AluOpType.min)
            nc.vector.tensor_tensor(out=t1, in0=t1, in1=t2,
                                    op=mybir.AluOpType.min)
            mpx = pxp.tile([128, 512], F32, tag="mx", name="mx")
            mx4 = mpx[0:4, 0:128]
            nc.tensor.transpose(mx4, t1, identity)
            mrow = mxsb.tile([4, 128], F16, tag="mrow", name="mrow", bufs=2)
            nc.scalar.activation(out=mrow, in_=mx4,
                                 func=mybir.ActivationFunctionType.Copy)
            nc.sync.dma_start(
                out=q_aug[D:D + 1, ic * 512:(ic + 1) * 512].rearrange(
                    "p (c f) -> p c f", c=4
                ),
                in_=mrow,
            )

        # ---- Phase 1: projections, with head 0's max pass woven in ----
        # ff order (0,2,1,3): heads 0/1 need qk tiles 0 and 2 first.
        # PSUM->SBUF copies go via ACT so the DVE is free for max reduces.
        with (
            tc.tile_pool(name="ph1", bufs=1) as ph1,
            tc.tile_pool(name="pj", bufs=4, space="PSUM") as pj,
            tc.tile_pool(name="pv", bufs=1, space="PSUM") as pv,
        ):
            xt_sb = [ph1.tile([128, T], F16, tag=f"xt{i}", name=f"xt{i}") for i in range(NE)]
            wqk_sb = [ph1.tile([128, FQK], F16, tag=f"wqk{i}", name=f"wqk{i}") for i in range(NE)]
            wv_sb = [ph1.tile([128, DV], F16, tag=f"wv{i}", name=f"wv{i}") for i in range(NE)]
            # two DMA queues: weights on SP, x/wv on the ACT queue, so the
            # first projection's operands arrive in parallel.
            for i in range(NE):
                nc.sync.dma_start(out=wqk_sb[i], in_=wqkT_d[i * 128:(i + 1) * 128, :])
                nc.scalar.dma_start(out=xt_sb[i], in_=xT_d[i * 128:(i + 1) * 128, :])
            for i in range(NE):
                nc.scalar.dma_start(out=wv_sb[i], in_=wvT_d[i * 128:(i + 1) * 128, :])

            def qk_proj(ff):
                # qk^T [f', t] = W'[f', e] @ x^T[e, t], feature-major
                ps = [pj.tile([128, 512], F32, tag="pj", name="pj") for _ in range(NT_CHUNK)]
                for ne in range(NE):
                    lhsT = wqk_sb[ne][:, ff * 128:(ff + 1) * 128]
                    for tt in range(NT_CHUNK):
                        nc.tensor.matmul(
                            ps[tt],
                            lhsT,
                            xt_sb[ne][:, tt * 512:(tt + 1) * 512],
                            start=(ne == 0),
                            stop=(ne == NE - 1),
                        )
                for tt in range(NT_CHUNK):
                    nc.scalar.activation(
                        out=qk_sb[ff][:, tt * 512:(tt + 1) * 512], in_=ps[tt],
                        func=mybir.ActivationFunctionType.Copy,
                    )

            qk_proj(0)
            qk_proj(2)
            build_qk_aug(0)
            build_qk_aug(1)
            for it in range(0, 4):
                max_it(0, it)
            max_finish(0, 0)
            qk_proj(1)
            for it in range(4, 8):
                max_it(0, it)
            max_finish(0, 1)
            qk_proj(3)
            for it in range(8, 12):
                max_it(0, it)
            max_finish(0, 2)

            # v [t, dv] token-major, two tiles per full-bank PSUM buffer
            # (half-bank tiles measured pathologically slow); head 0's
            # remaining max chunks weave between pairs.
            for tp in range(NT_TILE // 2):
                psv = pv.tile([128, 512], F32, tag="pv", name="pv")
                for half in range(2):
                    tj = tp * 2 + half
                    for ne in range(NE):
                        nc.tensor.matmul(
                            psv[:, half * DV:(half + 1) * DV],
                            xt_sb[ne][:, tj * 128:(tj + 1) * 128],
                            wv_sb[ne],
                            start=(ne == 0),
                            stop=(ne == NE - 1),
                        )
                nc.scalar.activation(
                    out=v_sb[:, tp * 2 * DV:(tp + 1) * 2 * DV], in_=psv,
                    func=mybir.ActivationFunctionType.Copy,
                )
                if tp in (1, 3, 5, 7):
                    max_it(0, 12 + tp // 2)
                    if tp == 7:
                        max_finish(0, 3)


        for h in range(HPG):
            odd = h % 2 == 1
            # V-stationary [128, 16*128] bf16: per j-tile 128 columns
            # (even) [v(64) | ones | 0...]   -> O rows 0..63, denom row 64
            # (odd)  [ones | 0... | v(64)]   -> denom row 0, O rows 64..127
            vaug = vaugp.tile([128, NT_TILE * 128], F16, tag=f"vaug{h}", name=f"vaug{h}")
            va_r = vaug.rearrange("p (j c) -> p j c", c=128)
            v_r = v_sb.rearrange("p (j c) -> p j c", c=DV)
            eng = nc.gpsimd if odd else nc.vector
            if odd:
                eng.memset(va_r[:, :, 0:1], 1.0)
                eng.memset(va_r[:, :, 1:D], 0.0)
                eng.tensor_copy(out=va_r[:, :, D:2 * D], in_=v_r[:, :, h * D:(h + 1) * D])
            else:
                eng.tensor_copy(out=va_r[:, :, 0:D], in_=v_r[:, :, h * D:(h + 1) * D])
                eng.memset(va_r[:, :, D:D + 1], 1.0)
                eng.memset(va_r[:, :, D + 1:128], 0.0)
            vaugs.append(vaug)

        # ---- Phase 2 main + phase 3 ------------------------------------
        with (
            tc.tile_pool(name="pt", bufs=3) as ptp,
            tc.tile_pool(name="rr", bufs=1) as rrp,
            tc.tile_pool(name="ot", bufs=2) as otp,
            tc.tile_pool(name="ysb", bufs=3) as ysbp,
            tc.tile_pool(name="ps", bufs=2, space="PSUM") as psp,
            tc.tile_pool(name="po", bufs=1, space="PSUM") as pop,
        ):
            # per-head persistent r tiles, zeroed once: the broadcast
            # matmul streams all 128 rows of r, so the 0-weighted rows must
            # hold finite values; only the denominator row is ever rewritten.
            r_tiles = []
            for h in range(HPG):
                rt = rrp.tile([128, 512], F16, tag=f"r{h}", name=f"r{h}", bufs=1)
                nc.vector.memset(rt, 0.0)
                r_tiles.append(rt)

            pending = []  # delayed (PE broadcast + DVE mul) closures

            def flush_pending():
                while pending:
                    pending.pop(0)()

            def phase3_tt(tt):
                # out-projection for one finished token tile; PSUM borrowed
                # from the (idle by now) max-pass pool pxp.
                yt = ysbp.tile([128, E], F32, tag="y", name="y")
                for oc in range(2):
                    pys = pxp.tile([128, 512], F32, tag="mx", name="mx")
                    for es in range(DV // 128):
                        nc.tensor.matmul(
                            pys,
                            oall_sb[es][:, tt * 128:(tt + 1) * 128],
                            wout_sb[es][:, oc * 512:(oc + 1) * 512],
                            start=(es == 0),
                            stop=(es == DV // 128 - 1),
                        )
                    nc.vector.tensor_copy(
                        out=yt[:, oc * 512:(oc + 1) * 512], in_=pys
                    )
                nc.sync.dma_start(out=y_d[tt * 128:(tt + 1) * 128, :], in_=yt)

            def phase3(ic):
                for tt in range(ic * NT_CHUNK, (ic + 1) * NT_CHUNK):
                    phase3_tt(tt)

            def main_block(h, ic, weave=(), weave_p3=None):
                odd = h % 2 == 1
                o_base = (h % 2) * D
                den = D if not odd else 0
                ics = slice(ic * 512, (ic + 1) * 512)
                q_aug, k_aug, vaug = q_augs[h], k_augs[h], vaugs[h]
                if weave_p3 is not None:
                    # the woven out-projection reads oall written by the
                    # still-pending previous normalize -- flush it first
                    flush_pending()
                po = pop.tile([128, 512], F32, tag="po", name="po")
                wits = [(wh, wc * NT_CHUNK + i) for wh, wc in weave
                        for i in range(NT_CHUNK)]
                per_hook = (len(wits) + 3) // 4 if wits else 0
                for jp in range(8):
                    if jp % 2 == 0:
                        jq = jp // 2
                        for wh, wit in wits[jq * per_hook:(jq + 1) * per_hook]:
                            max_it(wh, wit)
                        if weave_p3 is not None:
                            phase3_tt(weave_p3 * NT_CHUNK + jq)
                    ps = psp.tile([128, 1024], F32, tag="ps", name="ps")
                    for s in range(2):
                        jt = jp * 2 + s
                        nc.tensor.matmul(
                            ps[:, s * 512:(s + 1) * 512],
                            k_aug[:, jt * 128:(jt + 1) * 128],
                            q_aug[:, ics],
                            start=True,
                            stop=True,
                        )
                    pT = ptp.tile([128, 1024], F16, tag="pt", name="pt")
                    nc.scalar.activation(
                        out=pT, in_=ps, func=mybir.ActivationFunctionType.Exp
                    )
                    for s in range(2):
                        jt = jp * 2 + s
                        nc.tensor.matmul(
                            po,
                            vaug[:, jt * 128:(jt + 1) * 128],
                            pT[:, s * 512:(s + 1) * 512],
                            start=(jt == 0),
                            stop=(jt == NT_TILE - 1),
                        )
                for wh, wc in weave:
                    max_finish(wh, wc)
                flush_pending()
                # eager: one full-height DVE copy frees po's bank fast; the
                # denominator row rides along so the deferred normalize (and
                # its Ln/Exp on ACT) never touches PSUM, landing in the ACT
                # queue behind the next block's first exps instead of
                # head-of-line blocking them.
                ot = otp.tile([128, 512], F32, tag="ot", name="ot")
                nc.scalar.activation(out=ot, in_=po,
                                     func=mybir.ActivationFunctionType.Copy)

                def normalize():
                    # deferred one block: Ln/Exp land behind the next block's
                    # first exps in the ACT queue; the row broadcast and the
                    # final multiply run on the otherwise-idle gpsimd.
                    r = r_tiles[h]
                    lnt = otp.tile([128, 512], F32, tag="lnt", name="lnt", bufs=2)
                    nc.scalar.activation(
                        out=lnt[den:den + 1, :], in_=ot[den:den + 1, :],
                        func=mybir.ActivationFunctionType.Ln,
                    )
                    nc.scalar.activation(
                        out=r[den:den + 1, :], in_=lnt[den:den + 1, :],
                        func=mybir.ActivationFunctionType.Exp, scale=-1.0,
                    )
                    nc.sync.dma_start(
                        out=r[o_base:o_base + D, :],
                        in_=r[den:den + 1, None, :].broadcast_to([1, D, 512]),
                    )
                    nc.gpsimd.tensor_tensor(
                        out=oall_sb[h // 2][o_base:o_base + D, ics],
                        in0=ot[o_base:o_base + D, :],
                        in1=r[o_base:o_base + D, :],
                        op=mybir.AluOpType.mult,
                    )

                pending.append(normalize)
                if h == HPG - 1 and ic == NT_CHUNK - 1:
                    flush_pending()

            # ---- schedule ------------------------------------------------
            # heads 2/3's aug + fp8 builds are emitted mid-stream so their
            # gpsimd/DVE work lands in otherwise-idle weave slack, ready one
            # full head before their max pass starts.
            for h in range(HPG - 1):
                for ic in range(NT_CHUNK):
                    main_block(h, ic, weave=[(h + 1, ic)])
                    if ic == 1 and h < 2:
                        build_qk_aug(h + 2)
            for ic in range(NT_CHUNK):
                main_block(HPG - 1, ic, weave_p3=ic - 1 if ic > 0 else None)
            flush_pending()
            phase3(NT_CHUNK - 1)


def _build_nc(reps=1, debug=False, split_waits=True):
    nc = bass.Bass()
    xT_d = nc.declare_dram_parameter("xT", [E, T], F16, isOutput=False)
    wqkT_d = nc.declare_dram_parameter("wqkT", [E, FQK], F16, isOutput=False)
    wvT_d = nc.declare_dram_parameter("wvT", [E, DV], F16, isOutput=False)
    woutT_d = nc.declare_dram_parameter("woutT", [DV, E], F16, isOutput=False)
    y_d = nc.declare_dram_parameter("y", [T, E], F32, isOutput=True)
    dram = (xT_d, wqkT_d, wvT_d, woutT_d, y_d)
    dbg = None
    with tile_mod.TileContext(nc) as tc, nc.allow_low_precision(
        reason="fp16/bf16 kernel: scores/softmax accumulate in fp32 PSUM; the "
        "softmax shift comes from an fp8 max estimate within +-15 of the true "
        "max, absorbed by bf16 exp outputs; validated to rel-err ~5e-3 vs the "
        "fp64 reference"
    ):
        for _ in range(reps):
            with tc.tile_pool(name="persist", bufs=1) as persist:
                _emit_body(nc, tc, dram, {"persist": persist}, dbg=dbg)
    if split_waits:
        _split_multi_waits(nc)
    return nc


# ---------------------------------------------------------------------------
# Execution: cached jitted shard_map over 8 cores (axon/PJRT path)
_RUNNERS = {}


class _Runner:
    def __init__(self, reps=1, debug=False):
        import jax
        from jax.sharding import Mesh, PartitionSpec
        from jax.experimental.shard_map import shard_map
        from concourse import bass2jax

        bass2jax.install_neuronx_cc_hook()
        nc = self._nc = _build_nc(reps, debug=debug)

        partition_name = (
            nc.partition_id_tensor.name if nc.partition_id_tensor else None
        )
        in_names, out_names, out_avals, zero_outs = [], [], [], []
        for alloc in nc.m.functions[0].allocations:
            if not isinstance(alloc, mybir.MemoryLocationSet):
                continue
            name = alloc.memorylocations[0].name
            if alloc.kind == "ExternalInput":
                if name != partition_name:
                    in_names.append(name)
            elif alloc.kind == "ExternalOutput":
                shape = tuple(alloc.tensor_shape)
                dtype = mybir.dt.np(alloc.dtype)
                out_names.append(name)
                out_avals.append(jax.core.ShapedArray(shape, dtype))
                zero_outs.append(np.zeros(shape, dtype))
        self.in_names, self.out_names = in_names, out_names
        self.out_avals, self.zero_outs = out_avals, zero_outs
        n_params, n_outs = len(in_names), len(out_names)
        all_in_names = list(in_names) + list(out_names)
        if partition_name is not None:
            all_in_names.append(partition_name)
        all_in_names = tuple(all_in_names)

        def _body(*args):
            operands = list(args)
            if partition_name is not None:
                operands.append(bass2jax.partition_id_tensor())
            outs = bass2jax._bass_exec_p.bind(
                *operands,
                out_avals=tuple(out_avals),
                in_names=all_in_names,
                out_names=tuple(out_names),
                lowering_input_output_aliases=(),
                sim_require_finite=True,
                sim_require_nnan=True,
                nc=nc,
            )
            return tuple(outs)

        devices = jax.devices()[:N_CORES]
        assert len(devices) == N_CORES
        self.mesh = Mesh(np.asarray(devices), ("core",))
        in_specs = (PartitionSpec("core"),) * (n_params + n_outs)
        out_specs = (PartitionSpec("core"),) * n_outs
        self.donate = tuple(range(n_params, n_params + n_outs))
        self.sharded = jax.jit(
            shard_map(
                _body,
                mesh=self.mesh,
                in_specs=in_specs,
                out_specs=out_specs,
                check_rep=False,
            ),
            donate_argnums=self.donate,
            keep_unused=True,
        )

    def stage_inputs(self, per_core_in):
        """per_core_in: list of dicts (len N_CORES) -> device-resident concat arrays."""
        import jax
        from jax.sharding import NamedSharding, PartitionSpec

        sh = NamedSharding(self.mesh, PartitionSpec("core"))
        staged = []
        for name in self.in_names:
            cat = np.concatenate(
                [np.asarray(per_core_in[c][name]) for c in range(N_CORES)], axis=0
            )
            staged.append(jax.device_put(cat, sh))
        return staged

    def fresh_outs(self):
        import jax
        from jax.sharding import NamedSharding, PartitionSpec

        sh = NamedSharding(self.mesh, PartitionSpec("core"))
        return [
            jax.device_put(
                np.zeros((N_CORES * z.shape[0], *z.shape[1:]), z.dtype), sh
            )
            for z in self.zero_outs
        ]

    def run(self, staged_in, out_bufs):
        import jax

        outs = self.sharded(*staged_in, *out_bufs)
        jax.block_until_ready(outs)
        return outs

    def results(self, outs):
        res = []
        for c in range(N_CORES):
            d = {}
            for i, name in enumerate(self.out_names):
                full = np.asarray(outs[i])
                d[name] = full.reshape(N_CORES, *self.out_avals[i].shape)[c]
            res.append(d)
        return res


def _get_runner(reps=1):
    if reps not in _RUNNERS:
        _RUNNERS[reps] = _Runner(reps)
    return _RUNNERS[reps]


# ---------------------------------------------------------------------------
# Host-side sharding / gather
def _per_core_inputs(x, w_qkv, w_out):
    x = np.asarray(x, dtype=np.float32)
    w_qkv = np.asarray(w_qkv, dtype=np.float32)
    w_out = np.asarray(w_out, dtype=np.float32)
    per_core = []
    for c in range(N_CORES):
        b, g = c // GROUPS, c % GROUPS
        hs = np.arange(g * HPG, (g + 1) * HPG)
        # qkv reshape order in reference: f = d*48 + k*16 + h
        rows_q = (np.arange(D)[None, :] * (3 * H_TOTAL) + hs[:, None]).reshape(-1)
        rows_k = rows_q + H_TOTAL
        rows_v = rows_q + 2 * H_TOTAL
        wqk = np.concatenate([w_qkv[rows_q], SCALE * w_qkv[rows_k]], axis=0)
        per_core.append(
            {
                "xT": np.ascontiguousarray(x[b].T).astype(np.float16),
                "wqkT": np.ascontiguousarray(wqk.T).astype(np.float16),
                "wvT": np.ascontiguousarray(w_qkv[rows_v].T).astype(np.float16),
                "woutT": np.ascontiguousarray(w_out[:, g * DV:(g + 1) * DV].T).astype(np.float16),
            }
        )
    return per_core


def kernel(x, w_qkv, w_out):
    runner = _get_runner(1)
    staged = runner.stage_inputs(_per_core_inputs(x, w_qkv, w_out))
    outs = runner.run(staged, runner.fresh_outs())
    res = runner.results(outs)
    y = np.zeros((B, T, E), dtype=np.float64)
    for c in range(N_CORES):
        y[c // GROUPS] += res[c]["y"].astype(np.float64)
    return y.astype(np.float32)


# revision 30
# speedup vs baseline: 1.0178x; 1.0178x over previous
"""Trainium2 Bass kernel for 16-head MHA (B=2, T=2048, E=1024), SPMD on 8 cores.

Sharding: data-parallel over batch (2) x tensor-parallel over heads (4 groups
of 4 heads). Each core computes, for its (batch b, head-group g):
  qk^T projection (feature-major), v projection (token-major),
  shifted-softmax attention via an augmented-row matmul trick, and a partial
  output projection over its 256 embedding columns. The host sums the 4
  partial projections per batch.

Softmax shift: an approximate per-query max over all keys is computed
on-device from an fp8e4m3 copy of q/k using DoubleRow-mode matmuls (K=256
contraction at 0.5 cycles/row -> half the PE cost of the fp16 equivalent) in
[i, j] orientation + wide free-dim DVE reduce_max(negate=True). The estimate
is within +-15 of the true max (measured on N(0,1) data), so exp outputs are
bounded by e^15: pT is bf16 (range e^+-88) and the fp32 PSUM accumulations
are safe. The shift is folded into the main QK^T matmul as a rank-1
augmented row, so scores arrive in PSUM already shifted:
S'[j,i] = 8*q_i.k_j - M_i. exp() runs on ACT straight out of PSUM over
[128,1024] two-bank tiles. The softmax denominator comes for free from a
ones-column appended to V.
"""

import sys

sys.path.insert(0, "/opt/trn_rl_repo")

import numpy as np

import concourse.bass as bass
import concourse.mybir as mybir
import concourse.tile as tile_mod
from concourse.masks import make_identity

F32 = mybir.dt.float32
F16 = mybir.dt.float16
BF16 = mybir.dt.bfloat16
F8 = mybir.dt.float8e4

B, T, E = 2, 2048, 1024
H_TOTAL, D = 16, 64
N_CORES = 8
GROUPS = 4          # head-group (tensor) parallelism
HPG = H_TOTAL // GROUPS  # 4 heads per group
DV = HPG * D        # 256: v width / out-proj contraction per core
FQK = 2 * DV        # 512: q+k feature rows per core
SCALE = float(np.sqrt(D))  # reference MULTIPLIES scores by sqrt(d)

NE = E // 128       # 8 e-chunks
NT_TILE = T // 128  # 16 token tiles
NT_CHUNK = T // 512  # 4 token chunks


# ---------------------------------------------------------------------------
# Workaround: this walrus build only accepts ONE sem wait per instruction.
# After Tile scheduling, split every multi-wait instruction: the overflow
# waits move onto same-engine NoOps inserted immediately before it.
def _split_multi_waits(nc):
    for f in nc.m.functions:
        for bb in f.blocks:
            out = []
            for inst in bb.instructions:
                si = getattr(inst, "sync_info", None)
                if si is not None and si.on_wait and len(si.on_wait) > 1:
                    extras = list(si.on_wait[:-1])
                    si.on_wait = list(si.on_wait[-1:])
                    for w in extras:
                        nop = mybir.InstNoOp(
                            name=f"I-{nc.next_id()}", ins=[], outs=[]
                        )
                        nop.engine = inst.engine
                        nop.sync_info = mybir.SyncInfo(on_wait=[w], on_update=[])
                        out.append(nop)
                out.append(inst)
            bb.instructions[:] = out


# ---------------------------------------------------------------------------
# Device program (identical on every core; per-core data differs)
def _emit_body(nc, tc, dram, ctx_pools, dbg=None):
    xT_d, wqkT_d, wvT_d, woutT_d, y_d = dram
    persist = ctx_pools["persist"]

    # persistent SBUF
    qk_sb = [persist.tile([128, T], F16, tag=f"qk{i}", name=f"qk{i}") for i in range(FQK // 128)]
    # v as one [128, 16*256] tile: [t-tile partition, (jt, dv)] layout
    v_sb = persist.tile([128, NT_TILE * DV], F16, tag="v", name="v")
    oall_sb = [persist.tile([128, T], F16, tag=f"oall{i}", name=f"oall{i}") for i in range(DV // 128)]
    wout_sb = [persist.tile([128, E], F16, tag=f"wout{i}", name=f"wout{i}") for i in range(DV // 128)]
    identity = persist.tile([128, 128], F32, tag="identity", name="identity")
    make_identity(nc, identity)
    for i in range(DV // 128):
        nc.sync.dma_start(out=wout_sb[i], in_=woutT_d[i * 128:(i + 1) * 128, :])

    with (
        tc.tile_pool(name="aug", bufs=1) as augp,
        tc.tile_pool(name="vaug", bufs=1) as vaugp,
        tc.tile_pool(name="mx", bufs=1) as mxsb,
        tc.tile_pool(name="pmx", bufs=3, space="PSUM") as pxp,
    ):
        k_augs, q_augs, vaugs, m2s = [], [], [], []

        def build_qk_aug(h):
            # [128, T]: rows 0:64 = q|k, row 64 = -M slot (q, zero-init) /
            # ones (k), rows 65:128 = zero K-padding. Odd heads' rows live
            # at partitions 64..127 of qk_sb -> partition-shifting DMA.
            odd = h % 2 == 1
            q_tile, k_tile = h // 2, 2 + h // 2
            k_aug = augp.tile([128, T], F16, tag=f"kaug{h}", name=f"kaug{h}")
            q_aug = augp.tile([128, T], F16, tag=f"qaug{h}", name=f"qaug{h}")
            lo = D if odd else 0
            for cc in range(NT_CHUNK):
                cs = slice(cc * 512, (cc + 1) * 512)
                nc.sync.dma_start(out=k_aug[0:D, cs], in_=qk_sb[k_tile][lo:lo + D, cs])
                nc.sync.dma_start(out=q_aug[0:D, cs], in_=qk_sb[q_tile][lo:lo + D, cs])
            nc.gpsimd.memset(q_aug[D:128, :], 0.0)
            nc.gpsimd.memset(k_aug[D:128, :], 0.0)
            nc.gpsimd.memset(k_aug[D:D + 1, :], 1.0)
            k_augs.append(k_aug)
            q_augs.append(q_aug)

        # ---- max-pass emitters ----------------------------------------
        # fp16 scores in [i-part, j-free] orientation (q-stationary,
        # K=128 zero-padded); row max = narrow negated free-axis reduces.
        # q_aug row D is still zero when head h's own max pass runs, so
        # the augmented row contributes nothing here.
        def max_it(h, it):
            # one query tile: 4 matmuls + 4 narrow negated reduces
            if it == 0:
                m2 = mxsb.tile([128, NT_TILE * 4], F32, tag="m2", name="m2", bufs=2)
                m2s.append(m2)
            m2r = m2s[h].rearrange("p (j i) -> p j i", i=NT_TILE)
            q_aug, k_aug = q_augs[h], k_augs[h]
            for jc in range(4):
                ps = pxp.tile([128, 512], F32, tag="mx", name="mx")
                nc.tensor.matmul(
                    ps,
                    q_aug[:, it * 128:(it + 1) * 128],
                    k_aug[:, jc * 512:(jc + 1) * 512],
                    start=True,
                    stop=True,
                )
                nc.vector.reduce_max(
                    out=m2r[:, jc, it:it + 1], in_=ps,
                    axis=mybir.AxisListType.X, negate=True,
                )

        def max_finish(h, ic):
            # combine quarters for one 4-it chunk (values are negated:
            # min == -max), transpose to a row, DMA into q_aug row D.
            q_aug = q_augs[h]
            i0 = ic * NT_CHUNK
            m2r = m2s[h].rearrange("p (j i) -> p j i", i=NT_TILE)
            t1 = mxsb.tile([128, 4], F32, tag="t1", name="t1", bufs=2)
            t2 = mxsb.tile([128, 4], F32, tag="t2", name="t2", bufs=2)
            nc.vector.tensor_tensor(out=t1, in0=m2r[:, 0, i0:i0 + 4],
                                    in1=m2r[:, 1, i0:i0 + 4],
                                    op=mybir.AluOpType.min)
            nc.vector.tensor_tensor(out=t2, in0=m2r[:, 2, i0:i0 + 4],
                                    in1=m2r[:, 3, i0:i0 + 4],
                                    op=mybir.AluOpType.min)
            nc.vector.tensor_tensor(out=t1, in0=t1, in1=t2,
                                    op=mybir.AluOpType.min)
            mpx = pxp.tile([128, 512], F32, tag="mx", name="mx")
            mx4 = mpx[0:4, 0:128]
            nc.tensor.transpose(mx4, t1, identity)
            mrow = mxsb.tile([4, 128], F16, tag="mrow", name="mrow", bufs=2)
            nc.scalar.activation(out=mrow, in_=mx4,
                                 func=mybir.ActivationFunctionType.Copy)
            nc.sync.dma_start(
                out=q_aug[D:D + 1, ic * 512:(ic + 1) * 512].rearrange(
                    "p (c f) -> p c f", c=4
                ),
                in_=mrow,
            )

        # ---- Phase 1: projections, with head 0's max pass woven in ----
        # ff order (0,2,1,3): heads 0/1 need qk tiles 0 and 2 first.
        # PSUM->SBUF copies go via ACT so the DVE is free for max reduces.
        with (
            tc.tile_pool(name="ph1", bufs=1) as ph1,
            tc.tile_pool(name="pj", bufs=4, space="PSUM") as pj,
            tc.tile_pool(name="pv", bufs=1, space="PSUM") as pv,
        ):
            xt_sb = [ph1.tile([128, T], F16, tag=f"xt{i}", name=f"xt{i}") for i in range(NE)]
            wqk_sb = [ph1.tile([128, FQK], F16, tag=f"wqk{i}", name=f"wqk{i}") for i in range(NE)]
            wv_sb = [ph1.tile([128, DV], F16, tag=f"wv{i}", name=f"wv{i}") for i in range(NE)]
            # two DMA queues: weights on SP, x/wv on the ACT queue, so the
            # first projection's operands arrive in parallel.
            for i in range(NE):
                nc.sync.dma_start(out=wqk_sb[i], in_=wqkT_d[i * 128:(i + 1) * 128, :])
                nc.scalar.dma_start(out=xt_sb[i], in_=xT_d[i * 128:(i + 1) * 128, :])
            for i in range(NE):
                nc.scalar.dma_start(out=wv_sb[i], in_=wvT_d[i * 128:(i + 1) * 128, :])

            def qk_proj(ff):
                # qk^T [f', t] = W'[f', e] @ x^T[e, t], feature-major
                ps = [pj.tile([128, 512], F32, tag="pj", name="pj") for _ in range(NT_CHUNK)]
                for ne in range(NE):
                    lhsT = wqk_sb[ne][:, ff * 128:(ff + 1) * 128]
                    for tt in range(NT_CHUNK):
                        nc.tensor.matmul(
                            ps[tt],
                            lhsT,
                            xt_sb[ne][:, tt * 512:(tt + 1) * 512],
                            start=(ne == 0),
                            stop=(ne == NE - 1),
                        )
                for tt in range(NT_CHUNK):
                    nc.scalar.activation(
                        out=qk_sb[ff][:, tt * 512:(tt + 1) * 512], in_=ps[tt],
                        func=mybir.ActivationFunctionType.Copy,
                    )

            qk_proj(0)
            qk_proj(2)
            build_qk_aug(0)
            build_qk_aug(1)
            for it in range(0, 4):
                max_it(0, it)
            max_finish(0, 0)
            qk_proj(1)
            for it in range(4, 8):
                max_it(0, it)
            max_finish(0, 1)
            qk_proj(3)
            for it in range(8, 12):
                max_it(0, it)
            max_finish(0, 2)

            # v [t, dv] token-major, two tiles per full-bank PSUM buffer
            # (half-bank tiles measured pathologically slow); head 0's
            # remaining max chunks weave between pairs.
            for tp in range(NT_TILE // 2):
                psv = pv.tile([128, 512], F32, tag="pv", name="pv")
                for half in range(2):
                    tj = tp * 2 + half
                    for ne in range(NE):
                        nc.tensor.matmul(
                            psv[:, half * DV:(half + 1) * DV],
                            xt_sb[ne][:, tj * 128:(tj + 1) * 128],
                            wv_sb[ne],
                            start=(ne == 0),
                            stop=(ne == NE - 1),
                        )
                nc.scalar.activation(
                    out=v_sb[:, tp * 2 * DV:(tp + 1) * 2 * DV], in_=psv,
                    func=mybir.ActivationFunctionType.Copy,
                )
                if tp in (1, 3, 5, 7):
                    max_it(0, 12 + tp // 2)
                    if tp == 7:
                        max_finish(0, 3)


        for h in range(HPG):
            odd = h % 2 == 1
            # V-stationary [128, 16*128] bf16: per j-tile 128 columns
            # (even) [v(64) | ones | 0...]   -> O rows 0..63, denom row 64
            # (odd)  [ones | 0... | v(64)]   -> denom row 0, O rows 64..127
            vaug = vaugp.tile([128, NT_TILE * 128], F16, tag=f"vaug{h}", name=f"vaug{h}")
            va_r = vaug.rearrange("p (j c) -> p j c", c=128)
            v_r = v_sb.rearrange("p (j c) -> p j c", c=DV)
            eng = nc.gpsimd if odd else nc.vector
            if odd:
                eng.memset(va_r[:, :, 0:1], 1.0)
                eng.memset(va_r[:, :, 1:D], 0.0)
                eng.tensor_copy(out=va_r[:, :, D:2 * D], in_=v_r[:, :, h * D:(h + 1) * D])
            else:
                eng.tensor_copy(out=va_r[:, :, 0:D], in_=v_r[:, :, h * D:(h + 1) * D])
                eng.memset(va_r[:, :, D:D + 1], 1.0)
                eng.memset(va_r[:, :, D + 1:128], 0.0)
            vaugs.append(vaug)

        # ---- Phase 2 main + phase 3 ------------------------------------
        with (
            tc.tile_pool(name="pt", bufs=3) as ptp,
            tc.tile_pool(name="rr", bufs=1) as rrp,
            tc.tile_pool(name="ot", bufs=2) as otp,
            tc.tile_pool(name="ysb", bufs=3) as ysbp,
            tc.tile_pool(name="ps", bufs=2, space="PSUM") as psp,
            tc.tile_pool(name="po", bufs=1, space="PSUM") as pop,
        ):
            # per-head persistent r tiles, zeroed once: the broadcast
            # matmul streams all 128 rows of r, so the 0-weighted rows must
            # hold finite values; only the denominator row is ever rewritten.
            r_tiles = []
            for h in range(HPG):
                rt = rrp.tile([128, 512], F16, tag=f"r{h}", name=f"r{h}", bufs=1)
                nc.vector.memset(rt, 0.0)
                r_tiles.append(rt)

            pending = []  # delayed (PE broadcast + DVE mul) closures

            def flush_pending():
                while pending:
                    pending.pop(0)()

            def phase3_tt(tt):
                # out-projection for one finished token tile; PSUM borrowed
                # from the (idle by now) max-pass pool pxp.
                yt = ysbp.tile([128, E], F32, tag="y", name="y")
                for oc in range(2):
                    pys = pxp.tile([128, 512], F32, tag="mx", name="mx")
                    for es in range(DV // 128):
                        nc.tensor.matmul(
                            pys,
                            oall_sb[es][:, tt * 128:(tt + 1) * 128],
                            wout_sb[es][:, oc * 512:(oc + 1) * 512],
                            start=(es == 0),
                            stop=(es == DV // 128 - 1),
                        )
                    nc.vector.tensor_copy(
                        out=yt[:, oc * 512:(oc + 1) * 512], in_=pys
                    )
                nc.sync.dma_start(out=y_d[tt * 128:(tt + 1) * 128, :], in_=yt)

            def phase3(ic):
                for tt in range(ic * NT_CHUNK, (ic + 1) * NT_CHUNK):
                    phase3_tt(tt)

            def main_block(h, ic, weave=(), weave_p3=None):
                odd = h % 2 == 1
                o_base = (h % 2) * D
                den = D if not odd else 0
                ics = slice(ic * 512, (ic + 1) * 512)
                q_aug, k_aug, vaug = q_augs[h], k_augs[h], vaugs[h]
                if weave_p3 is not None:
                    # the woven out-projection reads oall written by the
                    # still-pending previous normalize -- flush it first
                    flush_pending()
                po = pop.tile([128, 512], F32, tag="po", name="po")
                wits = [(wh, wc * NT_CHUNK + i) for wh, wc in weave
                        for i in range(NT_CHUNK)]
                per_hook = (len(wits) + 3) // 4 if wits else 0
                for jp in range(8):
                    if jp % 2 == 0:
                        jq = jp // 2
                        for wh, wit in wits[jq * per_hook:(jq + 1) * per_hook]:
                            max_it(wh, wit)
                        if weave_p3 is not None:
                            phase3_tt(weave_p3 * NT_CHUNK + jq)
                    ps = psp.tile([128, 1024], F32, tag="ps", name="ps")
                    for s in range(2):
                        jt = jp * 2 + s
                        nc.tensor.matmul(
                            ps[:, s * 512:(s + 1) * 512],
                            k_aug[:, jt * 128:(jt + 1) * 128],
                            q_aug[:, ics],
                            start=True,
                            stop=True,
                        )
                    pT = ptp.tile([128, 1024], F16, tag="pt", name="pt")
                    nc.scalar.activation(
                        out=pT, in_=ps, func=mybir.ActivationFunctionType.Exp
                    )
                    for s in range(2):
                        jt = jp * 2 + s
                        nc.tensor.matmul(
                            po,
                            vaug[:, jt * 128:(jt + 1) * 128],
                            pT[:, s * 512:(s + 1) * 512],
                            start=(jt == 0),
                            stop=(jt == NT_TILE - 1),
                        )
                for wh, wc in weave:
                    max_finish(wh, wc)
                flush_pending()
                # eager: one full-height DVE copy frees po's bank fast; the
                # denominator row rides along so the deferred normalize (and
                # its Ln/Exp on ACT) never touches PSUM, landing in the ACT
                # queue behind the next block's first exps instead of
                # head-of-line blocking them.
                ot = otp.tile([128, 512], F32, tag="ot", name="ot")
                nc.scalar.activation(out=ot, in_=po,
                                     func=mybir.ActivationFunctionType.Copy)

                def normalize():
                    # deferred one block: Ln/Exp land behind the next block's
                    # first exps in the ACT queue; the row broadcast and the
                    # final multiply run on the otherwise-idle gpsimd.
                    r = r_tiles[h]
                    lnt = otp.tile([128, 512], F32, tag="lnt", name="lnt", bufs=2)
                    nc.scalar.activation(
                        out=lnt[den:den + 1, :], in_=ot[den:den + 1, :],
                        func=mybir.ActivationFunctionType.Ln,
                    )
                    nc.scalar.activation(
                        out=r[den:den + 1, :], in_=lnt[den:den + 1, :],
                        func=mybir.ActivationFunctionType.Exp, scale=-1.0,
                    )
                    nc.sync.dma_start(
                        out=r[o_base:o_base + D, :],
                        in_=r[den:den + 1, None, :].broadcast_to([1, D, 512]),
                    )
                    nc.gpsimd.tensor_tensor(
                        out=oall_sb[h // 2][o_base:o_base + D, ics],
                        in0=ot[o_base:o_base + D, :],
                        in1=r[o_base:o_base + D, :],
                        op=mybir.AluOpType.mult,
                    )

                pending.append(normalize)
                if h == HPG - 1:
                    # head 3: the next block's woven out-projection reads
                    # oall immediately -- don't defer the normalize chain.
                    flush_pending()

            # ---- schedule ------------------------------------------------
            # heads 2/3's aug + fp8 builds are emitted mid-stream so their
            # gpsimd/DVE work lands in otherwise-idle weave slack, ready one
            # full head before their max pass starts.
            for h in range(HPG - 1):
                for ic in range(NT_CHUNK):
                    main_block(h, ic, weave=[(h + 1, ic)])
                    if ic == 1 and h < 2:
                        build_qk_aug(h + 2)
            for ic in range(NT_CHUNK):
                main_block(HPG - 1, ic, weave_p3=ic - 1 if ic > 0 else None)
            flush_pending()
            phase3(NT_CHUNK - 1)


def _build_nc(reps=1, debug=False, split_waits=True):
    nc = bass.Bass()
    xT_d = nc.declare_dram_parameter("xT", [E, T], F16, isOutput=False)
    wqkT_d = nc.declare_dram_parameter("wqkT", [E, FQK], F16, isOutput=False)
    wvT_d = nc.declare_dram_parameter("wvT", [E, DV], F16, isOutput=False)
    woutT_d = nc.declare_dram_parameter("woutT", [DV, E], F16, isOutput=False)
    y_d = nc.declare_dram_parameter("y", [T, E], F32, isOutput=True)
    dram = (xT_d, wqkT_d, wvT_d, woutT_d, y_d)
    dbg = None
    with tile_mod.TileContext(nc) as tc, nc.allow_low_precision(
        reason="fp16/bf16 kernel: scores/softmax accumulate in fp32 PSUM; the "
        "softmax shift comes from an fp8 max estimate within +-15 of the true "
        "max, absorbed by bf16 exp outputs; validated to rel-err ~5e-3 vs the "
        "fp64 reference"
    ):
        for _ in range(reps):
            with tc.tile_pool(name="persist", bufs=1) as persist:
                _emit_body(nc, tc, dram, {"persist": persist}, dbg=dbg)
    if split_waits:
        _split_multi_waits(nc)
    return nc


# ---------------------------------------------------------------------------
# Execution: cached jitted shard_map over 8 cores (axon/PJRT path)
_RUNNERS = {}


class _Runner:
    def __init__(self, reps=1, debug=False):
        import jax
        from jax.sharding import Mesh, PartitionSpec
        from jax.experimental.shard_map import shard_map
        from concourse import bass2jax

        bass2jax.install_neuronx_cc_hook()
        nc = self._nc = _build_nc(reps, debug=debug)

        partition_name = (
            nc.partition_id_tensor.name if nc.partition_id_tensor else None
        )
        in_names, out_names, out_avals, zero_outs = [], [], [], []
        for alloc in nc.m.functions[0].allocations:
            if not isinstance(alloc, mybir.MemoryLocationSet):
                continue
            name = alloc.memorylocations[0].name
            if alloc.kind == "ExternalInput":
                if name != partition_name:
                    in_names.append(name)
            elif alloc.kind == "ExternalOutput":
                shape = tuple(alloc.tensor_shape)
                dtype = mybir.dt.np(alloc.dtype)
                out_names.append(name)
                out_avals.append(jax.core.ShapedArray(shape, dtype))
                zero_outs.append(np.zeros(shape, dtype))
        self.in_names, self.out_names = in_names, out_names
        self.out_avals, self.zero_outs = out_avals, zero_outs
        n_params, n_outs = len(in_names), len(out_names)
        all_in_names = list(in_names) + list(out_names)
        if partition_name is not None:
            all_in_names.append(partition_name)
        all_in_names = tuple(all_in_names)

        def _body(*args):
            operands = list(args)
            if partition_name is not None:
                operands.append(bass2jax.partition_id_tensor())
            outs = bass2jax._bass_exec_p.bind(
                *operands,
                out_avals=tuple(out_avals),
                in_names=all_in_names,
                out_names=tuple(out_names),
                lowering_input_output_aliases=(),
                sim_require_finite=True,
                sim_require_nnan=True,
                nc=nc,
            )
            return tuple(outs)

        devices = jax.devices()[:N_CORES]
        assert len(devices) == N_CORES
        self.mesh = Mesh(np.asarray(devices), ("core",))
        in_specs = (PartitionSpec("core"),) * (n_params + n_outs)
        out_specs = (PartitionSpec("core"),) * n_outs
        self.donate = tuple(range(n_params, n_params + n_outs))
        self.sharded = jax.jit(
            shard_map(
                _body,
                mesh=self.mesh,
                in_specs=in_specs,
                out_specs=out_specs,
                check_rep=False,
            ),
            donate_argnums=self.donate,
            keep_unused=True,
        )

    def stage_inputs(self, per_core_in):
        """per_core_in: list of dicts (len N_CORES) -> device-resident concat arrays."""
        import jax
        from jax.sharding import NamedSharding, PartitionSpec

        sh = NamedSharding(self.mesh, PartitionSpec("core"))
        staged = []
        for name in self.in_names:
            cat = np.concatenate(
                [np.asarray(per_core_in[c][name]) for c in range(N_CORES)], axis=0
            )
            staged.append(jax.device_put(cat, sh))
        return staged

    def fresh_outs(self):
        import jax
        from jax.sharding import NamedSharding, PartitionSpec

        sh = NamedSharding(self.mesh, PartitionSpec("core"))
        return [
            jax.device_put(
                np.zeros((N_CORES * z.shape[0], *z.shape[1:]), z.dtype), sh
            )
            for z in self.zero_outs
        ]

    def run(self, staged_in, out_bufs):
        import jax

        outs = self.sharded(*staged_in, *out_bufs)
        jax.block_until_ready(outs)
        return outs

    def results(self, outs):
        res = []
        for c in range(N_CORES):
            d = {}
            for i, name in enumerate(self.out_names):
                full = np.asarray(outs[i])
                d[name] = full.reshape(N_CORES, *self.out_avals[i].shape)[c]
            res.append(d)
        return res


def _get_runner(reps=1):
    if reps not in _RUNNERS:
        _RUNNERS[reps] = _Runner(reps)
    return _RUNNERS[reps]


# ---------------------------------------------------------------------------
# Host-side sharding / gather
def _per_core_inputs(x, w_qkv, w_out):
    x = np.asarray(x, dtype=np.float32)
    w_qkv = np.asarray(w_qkv, dtype=np.float32)
    w_out = np.asarray(w_out, dtype=np.float32)
    per_core = []
    for c in range(N_CORES):
        b, g = c // GROUPS, c % GROUPS
        hs = np.arange(g * HPG, (g + 1) * HPG)
        # qkv reshape order in reference: f = d*48 + k*16 + h
        rows_q = (np.arange(D)[None, :] * (3 * H_TOTAL) + hs[:, None]).reshape(-1)
        rows_k = rows_q + H_TOTAL
        rows_v = rows_q + 2 * H_TOTAL
        wqk = np.concatenate([w_qkv[rows_q], SCALE * w_qkv[rows_k]], axis=0)
        per_core.append(
            {
                "xT": np.ascontiguousarray(x[b].T).astype(np.float16),
                "wqkT": np.ascontiguousarray(wqk.T).astype(np.float16),
                "wvT": np.ascontiguousarray(w_qkv[rows_v].T).astype(np.float16),
                "woutT": np.ascontiguousarray(w_out[:, g * DV:(g + 1) * DV].T).astype(np.float16),
            }
        )
    return per_core


def kernel(x, w_qkv, w_out):
    runner = _get_runner(1)
    staged = runner.stage_inputs(_per_core_inputs(x, w_qkv, w_out))
    outs = runner.run(staged, runner.fresh_outs())
    res = runner.results(outs)
    y = np.zeros((B, T, E), dtype=np.float64)
    for c in range(N_CORES):
        y[c // GROUPS] += res[c]["y"].astype(np.float64)
    return y.astype(np.float32)


# revision 31
# speedup vs baseline: 1.0245x; 1.0066x over previous
"""Trainium2 Bass kernel for 16-head MHA (B=2, T=2048, E=1024), SPMD on 8 cores.

Sharding: data-parallel over batch (2) x tensor-parallel over heads (4 groups
of 4 heads). Each core computes, for its (batch b, head-group g):
  qk^T projection (feature-major), v projection (token-major),
  shifted-softmax attention via an augmented-row matmul trick, and a partial
  output projection over its 256 embedding columns. The host sums the 4
  partial projections per batch.

Softmax shift: an approximate per-query max over all keys is computed
on-device from an fp8e4m3 copy of q/k using DoubleRow-mode matmuls (K=256
contraction at 0.5 cycles/row -> half the PE cost of the fp16 equivalent) in
[i, j] orientation + wide free-dim DVE reduce_max(negate=True). The estimate
is within +-15 of the true max (measured on N(0,1) data), so exp outputs are
bounded by e^15: pT is bf16 (range e^+-88) and the fp32 PSUM accumulations
are safe. The shift is folded into the main QK^T matmul as a rank-1
augmented row, so scores arrive in PSUM already shifted:
S'[j,i] = 8*q_i.k_j - M_i. exp() runs on ACT straight out of PSUM over
[128,1024] two-bank tiles. The softmax denominator comes for free from a
ones-column appended to V.
"""

import sys

sys.path.insert(0, "/opt/trn_rl_repo")

import numpy as np

import concourse.bass as bass
import concourse.mybir as mybir
import concourse.tile as tile_mod
from concourse.masks import make_identity

F32 = mybir.dt.float32
F16 = mybir.dt.float16
BF16 = mybir.dt.bfloat16
F8 = mybir.dt.float8e4

B, T, E = 2, 2048, 1024
H_TOTAL, D = 16, 64
N_CORES = 8
GROUPS = 4          # head-group (tensor) parallelism
HPG = H_TOTAL // GROUPS  # 4 heads per group
DV = HPG * D        # 256: v width / out-proj contraction per core
FQK = 2 * DV        # 512: q+k feature rows per core
SCALE = float(np.sqrt(D))  # reference MULTIPLIES scores by sqrt(d)

NE = E // 128       # 8 e-chunks
NT_TILE = T // 128  # 16 token tiles
NT_CHUNK = T // 512  # 4 token chunks


# ---------------------------------------------------------------------------
# Workaround: this walrus build only accepts ONE sem wait per instruction.
# After Tile scheduling, split every multi-wait instruction: the overflow
# waits move onto same-engine NoOps inserted immediately before it.
def _split_multi_waits(nc):
    for f in nc.m.functions:
        for bb in f.blocks:
            out = []
            for inst in bb.instructions:
                si = getattr(inst, "sync_info", None)
                if si is not None and si.on_wait and len(si.on_wait) > 1:
                    extras = list(si.on_wait[:-1])
                    si.on_wait = list(si.on_wait[-1:])
                    for w in extras:
                        nop = mybir.InstNoOp(
                            name=f"I-{nc.next_id()}", ins=[], outs=[]
                        )
                        nop.engine = inst.engine
                        nop.sync_info = mybir.SyncInfo(on_wait=[w], on_update=[])
                        out.append(nop)
                out.append(inst)
            bb.instructions[:] = out


# ---------------------------------------------------------------------------
# Device program (identical on every core; per-core data differs)
def _emit_body(nc, tc, dram, ctx_pools, dbg=None):
    xT_d, wqkT_d, wvT_d, woutT_d, y_d = dram
    persist = ctx_pools["persist"]

    # persistent SBUF
    qk_sb = [persist.tile([128, T], F16, tag=f"qk{i}", name=f"qk{i}") for i in range(FQK // 128)]
    # v as one [128, 16*256] tile: [t-tile partition, (jt, dv)] layout
    v_sb = persist.tile([128, NT_TILE * DV], F16, tag="v", name="v")
    oall_sb = [persist.tile([128, T], F16, tag=f"oall{i}", name=f"oall{i}") for i in range(DV // 128)]
    wout_sb = [persist.tile([128, E], F16, tag=f"wout{i}", name=f"wout{i}") for i in range(DV // 128)]
    identity = persist.tile([128, 128], F32, tag="identity", name="identity")
    make_identity(nc, identity)
    for i in range(DV // 128):
        nc.sync.dma_start(out=wout_sb[i], in_=woutT_d[i * 128:(i + 1) * 128, :])

    with (
        tc.tile_pool(name="aug", bufs=1) as augp,
        tc.tile_pool(name="vaug", bufs=1) as vaugp,
        tc.tile_pool(name="mx", bufs=1) as mxsb,
        tc.tile_pool(name="pmx", bufs=3, space="PSUM") as pxp,
    ):
        k_augs, q_augs, vaugs, m2s = [], [], [], []

        def build_qk_aug(h):
            # [128, T]: rows 0:64 = q|k, row 64 = -M slot (q, zero-init) /
            # ones (k), rows 65:128 = zero K-padding. Odd heads' rows live
            # at partitions 64..127 of qk_sb -> partition-shifting DMA.
            odd = h % 2 == 1
            q_tile, k_tile = h // 2, 2 + h // 2
            k_aug = augp.tile([128, T], F16, tag=f"kaug{h}", name=f"kaug{h}")
            q_aug = augp.tile([128, T], F16, tag=f"qaug{h}", name=f"qaug{h}")
            lo = D if odd else 0
            for cc in range(NT_CHUNK):
                cs = slice(cc * 512, (cc + 1) * 512)
                nc.sync.dma_start(out=k_aug[0:D, cs], in_=qk_sb[k_tile][lo:lo + D, cs])
                nc.sync.dma_start(out=q_aug[0:D, cs], in_=qk_sb[q_tile][lo:lo + D, cs])
            nc.gpsimd.memset(q_aug[D:128, :], 0.0)
            nc.gpsimd.memset(k_aug[D:128, :], 0.0)
            nc.gpsimd.memset(k_aug[D:D + 1, :], 1.0)
            k_augs.append(k_aug)
            q_augs.append(q_aug)

        # ---- max-pass emitters ----------------------------------------
        # fp16 scores in [i-part, j-free] orientation (q-stationary,
        # K=128 zero-padded); row max = narrow negated free-axis reduces.
        # q_aug row D is still zero when head h's own max pass runs, so
        # the augmented row contributes nothing here.
        def max_it(h, it):
            # one query tile: 4 matmuls + 4 narrow negated reduces
            if it == 0:
                m2 = mxsb.tile([128, NT_TILE * 4], F32, tag="m2", name="m2", bufs=2)
                m2s.append(m2)
            m2r = m2s[h].rearrange("p (j i) -> p j i", i=NT_TILE)
            q_aug, k_aug = q_augs[h], k_augs[h]
            for jc in range(4):
                ps = pxp.tile([128, 512], F32, tag="mx", name="mx")
                nc.tensor.matmul(
                    ps,
                    q_aug[:, it * 128:(it + 1) * 128],
                    k_aug[:, jc * 512:(jc + 1) * 512],
                    start=True,
                    stop=True,
                )
                nc.vector.reduce_max(
                    out=m2r[:, jc, it:it + 1], in_=ps,
                    axis=mybir.AxisListType.X, negate=True,
                )

        def max_finish(h, ic):
            # combine quarters for one 4-it chunk (values are negated:
            # min == -max), transpose to a row, DMA into q_aug row D.
            q_aug = q_augs[h]
            i0 = ic * NT_CHUNK
            m2r = m2s[h].rearrange("p (j i) -> p j i", i=NT_TILE)
            t1 = mxsb.tile([128, 4], F32, tag="t1", name="t1", bufs=2)
            t2 = mxsb.tile([128, 4], F32, tag="t2", name="t2", bufs=2)
            nc.vector.tensor_tensor(out=t1, in0=m2r[:, 0, i0:i0 + 4],
                                    in1=m2r[:, 1, i0:i0 + 4],
                                    op=mybir.AluOpType.min)
            nc.vector.tensor_tensor(out=t2, in0=m2r[:, 2, i0:i0 + 4],
                                    in1=m2r[:, 3, i0:i0 + 4],
                                    op=mybir.AluOpType.min)
            nc.vector.tensor_tensor(out=t1, in0=t1, in1=t2,
                                    op=mybir.AluOpType.min)
            mpx = pxp.tile([128, 512], F32, tag="mx", name="mx")
            mx4 = mpx[0:4, 0:128]
            nc.tensor.transpose(mx4, t1, identity)
            mrow = mxsb.tile([4, 128], F16, tag="mrow", name="mrow", bufs=2)
            nc.scalar.activation(out=mrow, in_=mx4,
                                 func=mybir.ActivationFunctionType.Copy)
            nc.sync.dma_start(
                out=q_aug[D:D + 1, ic * 512:(ic + 1) * 512].rearrange(
                    "p (c f) -> p c f", c=4
                ),
                in_=mrow,
            )

        # ---- Phase 1: projections, with head 0's max pass woven in ----
        # ff order (0,2,1,3): heads 0/1 need qk tiles 0 and 2 first.
        # PSUM->SBUF copies go via ACT so the DVE is free for max reduces.
        with (
            tc.tile_pool(name="ph1", bufs=1) as ph1,
            tc.tile_pool(name="pj", bufs=4, space="PSUM") as pj,
            tc.tile_pool(name="pv", bufs=1, space="PSUM") as pv,
        ):
            xt_sb = [ph1.tile([128, T], F16, tag=f"xt{i}", name=f"xt{i}") for i in range(NE)]
            wqk_sb = [ph1.tile([128, FQK], F16, tag=f"wqk{i}", name=f"wqk{i}") for i in range(NE)]
            wv_sb = [ph1.tile([128, DV], F16, tag=f"wv{i}", name=f"wv{i}") for i in range(NE)]
            # two DMA queues: weights on SP, x/wv on the ACT queue, so the
            # first projection's operands arrive in parallel.
            for i in range(NE):
                nc.sync.dma_start(out=wqk_sb[i], in_=wqkT_d[i * 128:(i + 1) * 128, :])
                nc.scalar.dma_start(out=xt_sb[i], in_=xT_d[i * 128:(i + 1) * 128, :])
            for i in range(NE):
                nc.scalar.dma_start(out=wv_sb[i], in_=wvT_d[i * 128:(i + 1) * 128, :])

            def qk_proj(ff):
                # qk^T [f', t] = W'[f', e] @ x^T[e, t], feature-major
                ps = [pj.tile([128, 512], F32, tag="pj", name="pj") for _ in range(NT_CHUNK)]
                for ne in range(NE):
                    lhsT = wqk_sb[ne][:, ff * 128:(ff + 1) * 128]
                    for tt in range(NT_CHUNK):
                        nc.tensor.matmul(
                            ps[tt],
                            lhsT,
                            xt_sb[ne][:, tt * 512:(tt + 1) * 512],
                            start=(ne == 0),
                            stop=(ne == NE - 1),
                        )
                for tt in range(NT_CHUNK):
                    nc.scalar.activation(
                        out=qk_sb[ff][:, tt * 512:(tt + 1) * 512], in_=ps[tt],
                        func=mybir.ActivationFunctionType.Copy,
                    )

            qk_proj(0)
            qk_proj(2)
            build_qk_aug(0)
            build_qk_aug(1)
            for it in range(0, 4):
                max_it(0, it)
            max_finish(0, 0)
            qk_proj(1)
            for it in range(4, 8):
                max_it(0, it)
            max_finish(0, 1)
            qk_proj(3)
            for it in range(8, 12):
                max_it(0, it)
            max_finish(0, 2)

            # v [t, dv] token-major, two tiles per full-bank PSUM buffer
            # (half-bank tiles measured pathologically slow); head 0's
            # remaining max chunks weave between pairs.
            for tp in range(NT_TILE // 2):
                psv = pv.tile([128, 512], F32, tag="pv", name="pv")
                for half in range(2):
                    tj = tp * 2 + half
                    for ne in range(NE):
                        nc.tensor.matmul(
                            psv[:, half * DV:(half + 1) * DV],
                            xt_sb[ne][:, tj * 128:(tj + 1) * 128],
                            wv_sb[ne],
                            start=(ne == 0),
                            stop=(ne == NE - 1),
                        )
                nc.scalar.activation(
                    out=v_sb[:, tp * 2 * DV:(tp + 1) * 2 * DV], in_=psv,
                    func=mybir.ActivationFunctionType.Copy,
                )
                if tp in (1, 3, 5, 7):
                    max_it(0, 12 + tp // 2)
                    if tp == 7:
                        max_finish(0, 3)


        for h in range(HPG):
            odd = h % 2 == 1
            # V-stationary [128, 16*128] bf16: per j-tile 128 columns
            # (even) [v(64) | ones | 0...]   -> O rows 0..63, denom row 64
            # (odd)  [ones | 0... | v(64)]   -> denom row 0, O rows 64..127
            vaug = vaugp.tile([128, NT_TILE * 128], F16, tag=f"vaug{h}", name=f"vaug{h}")
            va_r = vaug.rearrange("p (j c) -> p j c", c=128)
            v_r = v_sb.rearrange("p (j c) -> p j c", c=DV)
            eng = nc.gpsimd if odd else nc.vector
            if odd:
                eng.memset(va_r[:, :, 0:1], 1.0)
                eng.memset(va_r[:, :, 1:D], 0.0)
                eng.tensor_copy(out=va_r[:, :, D:2 * D], in_=v_r[:, :, h * D:(h + 1) * D])
            else:
                eng.tensor_copy(out=va_r[:, :, 0:D], in_=v_r[:, :, h * D:(h + 1) * D])
                eng.memset(va_r[:, :, D:D + 1], 1.0)
                eng.memset(va_r[:, :, D + 1:128], 0.0)
            vaugs.append(vaug)

        # ---- Phase 2 main + phase 3 ------------------------------------
        with (
            tc.tile_pool(name="pt", bufs=3) as ptp,
            tc.tile_pool(name="rr", bufs=1) as rrp,
            tc.tile_pool(name="ot", bufs=2) as otp,
            tc.tile_pool(name="ysb", bufs=3) as ysbp,
            tc.tile_pool(name="ps", bufs=2, space="PSUM") as psp,
            tc.tile_pool(name="po", bufs=1, space="PSUM") as pop,
        ):
            # per-head persistent r tiles, zeroed once: the broadcast
            # matmul streams all 128 rows of r, so the 0-weighted rows must
            # hold finite values; only the denominator row is ever rewritten.
            r_tiles = []
            for h in range(HPG):
                rt = rrp.tile([128, 512], F16, tag=f"r{h}", name=f"r{h}", bufs=1)
                nc.vector.memset(rt, 0.0)
                r_tiles.append(rt)

            pending = []  # delayed (PE broadcast + DVE mul) closures

            def flush_pending():
                while pending:
                    pending.pop(0)()

            def phase3_tt(tt):
                # out-projection for one finished token tile; PSUM borrowed
                # from the (idle by now) max-pass pool pxp.
                yt = ysbp.tile([128, E], F32, tag="y", name="y")
                for oc in range(2):
                    pys = pxp.tile([128, 512], F32, tag="mx", name="mx")
                    for es in range(DV // 128):
                        nc.tensor.matmul(
                            pys,
                            oall_sb[es][:, tt * 128:(tt + 1) * 128],
                            wout_sb[es][:, oc * 512:(oc + 1) * 512],
                            start=(es == 0),
                            stop=(es == DV // 128 - 1),
                        )
                    nc.vector.tensor_copy(
                        out=yt[:, oc * 512:(oc + 1) * 512], in_=pys
                    )
                nc.sync.dma_start(out=y_d[tt * 128:(tt + 1) * 128, :], in_=yt)

            def phase3(ic):
                for tt in range(ic * NT_CHUNK, (ic + 1) * NT_CHUNK):
                    phase3_tt(tt)

            def main_block(h, ic, weave=(), weave_p3=None):
                odd = h % 2 == 1
                o_base = (h % 2) * D
                den = D if not odd else 0
                ics = slice(ic * 512, (ic + 1) * 512)
                q_aug, k_aug, vaug = q_augs[h], k_augs[h], vaugs[h]
                if weave_p3 is not None:
                    # the woven out-projection reads oall written by the
                    # still-pending previous normalize -- flush it first
                    flush_pending()
                po = pop.tile([128, 512], F32, tag="po", name="po")
                wits = [(wh, wc * NT_CHUNK + i) for wh, wc in weave
                        for i in range(NT_CHUNK)]
                per_hook = (len(wits) + 3) // 4 if wits else 0
                for jp in range(8):
                    if jp % 2 == 0:
                        jq = jp // 2
                        for wh, wit in wits[jq * per_hook:(jq + 1) * per_hook]:
                            max_it(wh, wit)
                        if weave_p3 is not None:
                            phase3_tt(weave_p3 * NT_CHUNK + jq)
                    ps = psp.tile([128, 1024], F32, tag="ps", name="ps")
                    for s in range(2):
                        jt = jp * 2 + s
                        nc.tensor.matmul(
                            ps[:, s * 512:(s + 1) * 512],
                            k_aug[:, jt * 128:(jt + 1) * 128],
                            q_aug[:, ics],
                            start=True,
                            stop=True,
                        )
                    pT = ptp.tile([128, 1024], F16, tag="pt", name="pt")
                    nc.scalar.activation(
                        out=pT, in_=ps, func=mybir.ActivationFunctionType.Exp
                    )
                    for s in range(2):
                        jt = jp * 2 + s
                        nc.tensor.matmul(
                            po,
                            vaug[:, jt * 128:(jt + 1) * 128],
                            pT[:, s * 512:(s + 1) * 512],
                            start=(jt == 0),
                            stop=(jt == NT_TILE - 1),
                        )
                for wh, wc in weave:
                    max_finish(wh, wc)
                flush_pending()
                # eager: one full-height DVE copy frees po's bank fast; the
                # denominator row rides along so the deferred normalize (and
                # its Ln/Exp on ACT) never touches PSUM, landing in the ACT
                # queue behind the next block's first exps instead of
                # head-of-line blocking them.
                ot = otp.tile([128, 512], F32, tag="ot", name="ot")
                nc.scalar.activation(out=ot, in_=po,
                                     func=mybir.ActivationFunctionType.Copy)

                def normalize():
                    # deferred one block: Ln/Exp land behind the next block's
                    # first exps in the ACT queue; the row broadcast and the
                    # final multiply run on the otherwise-idle gpsimd.
                    r = r_tiles[h]
                    lnt = otp.tile([128, 512], F32, tag="lnt", name="lnt", bufs=2)
                    nc.scalar.activation(
                        out=lnt[den:den + 1, :], in_=ot[den:den + 1, :],
                        func=mybir.ActivationFunctionType.Ln,
                    )
                    nc.scalar.activation(
                        out=r[den:den + 1, :], in_=lnt[den:den + 1, :],
                        func=mybir.ActivationFunctionType.Exp, scale=-1.0,
                    )
                    # head 3's chain is latency-critical (the next block's
                    # woven out-projection reads oall): issue its broadcast
                    # right behind Exp on the ACT queue and multiply on the
                    # (idle there) DVE; other heads keep sync-queue + gpsimd.
                    dma_eng = nc.scalar if h == HPG - 1 else nc.sync
                    mul_eng = nc.vector if h == HPG - 1 else nc.gpsimd
                    dma_eng.dma_start(
                        out=r[o_base:o_base + D, :],
                        in_=r[den:den + 1, None, :].broadcast_to([1, D, 512]),
                    )
                    mul_eng.tensor_tensor(
                        out=oall_sb[h // 2][o_base:o_base + D, ics],
                        in0=ot[o_base:o_base + D, :],
                        in1=r[o_base:o_base + D, :],
                        op=mybir.AluOpType.mult,
                    )

                pending.append(normalize)
                if h == HPG - 1:
                    # head 3: the next block's woven out-projection reads
                    # oall immediately -- don't defer the normalize chain.
                    flush_pending()

            # ---- schedule ------------------------------------------------
            # heads 2/3's aug + fp8 builds are emitted mid-stream so their
            # gpsimd/DVE work lands in otherwise-idle weave slack, ready one
            # full head before their max pass starts.
            for h in range(HPG - 1):
                for ic in range(NT_CHUNK):
                    main_block(h, ic, weave=[(h + 1, ic)])
                    if ic == 1 and h < 2:
                        build_qk_aug(h + 2)
            for ic in range(NT_CHUNK):
                main_block(HPG - 1, ic, weave_p3=ic - 1 if ic > 0 else None)
            flush_pending()
            phase3(NT_CHUNK - 1)


def _build_nc(reps=1, debug=False, split_waits=True):
    nc = bass.Bass()
    xT_d = nc.declare_dram_parameter("xT", [E, T], F16, isOutput=False)
    wqkT_d = nc.declare_dram_parameter("wqkT", [E, FQK], F16, isOutput=False)
    wvT_d = nc.declare_dram_parameter("wvT", [E, DV], F16, isOutput=False)
    woutT_d = nc.declare_dram_parameter("woutT", [DV, E], F16, isOutput=False)
    y_d = nc.declare_dram_parameter("y", [T, E], F32, isOutput=True)
    dram = (xT_d, wqkT_d, wvT_d, woutT_d, y_d)
    dbg = None
    with tile_mod.TileContext(nc) as tc, nc.allow_low_precision(
        reason="fp16/bf16 kernel: scores/softmax accumulate in fp32 PSUM; the "
        "softmax shift comes from an fp8 max estimate within +-15 of the true "
        "max, absorbed by bf16 exp outputs; validated to rel-err ~5e-3 vs the "
        "fp64 reference"
    ):
        for _ in range(reps):
            with tc.tile_pool(name="persist", bufs=1) as persist:
                _emit_body(nc, tc, dram, {"persist": persist}, dbg=dbg)
    if split_waits:
        _split_multi_waits(nc)
    return nc


# ---------------------------------------------------------------------------
# Execution: cached jitted shard_map over 8 cores (axon/PJRT path)
_RUNNERS = {}


class _Runner:
    def __init__(self, reps=1, debug=False):
        import jax
        from jax.sharding import Mesh, PartitionSpec
        from jax.experimental.shard_map import shard_map
        from concourse import bass2jax

        bass2jax.install_neuronx_cc_hook()
        nc = self._nc = _build_nc(reps, debug=debug)

        partition_name = (
            nc.partition_id_tensor.name if nc.partition_id_tensor else None
        )
        in_names, out_names, out_avals, zero_outs = [], [], [], []
        for alloc in nc.m.functions[0].allocations:
            if not isinstance(alloc, mybir.MemoryLocationSet):
                continue
            name = alloc.memorylocations[0].name
            if alloc.kind == "ExternalInput":
                if name != partition_name:
                    in_names.append(name)
            elif alloc.kind == "ExternalOutput":
                shape = tuple(alloc.tensor_shape)
                dtype = mybir.dt.np(alloc.dtype)
                out_names.append(name)
                out_avals.append(jax.core.ShapedArray(shape, dtype))
                zero_outs.append(np.zeros(shape, dtype))
        self.in_names, self.out_names = in_names, out_names
        self.out_avals, self.zero_outs = out_avals, zero_outs
        n_params, n_outs = len(in_names), len(out_names)
        all_in_names = list(in_names) + list(out_names)
        if partition_name is not None:
            all_in_names.append(partition_name)
        all_in_names = tuple(all_in_names)

        def _body(*args):
            operands = list(args)
            if partition_name is not None:
                operands.append(bass2jax.partition_id_tensor())
            outs = bass2jax._bass_exec_p.bind(
                *operands,
                out_avals=tuple(out_avals),
                in_names=all_in_names,
                out_names=tuple(out_names),
                lowering_input_output_aliases=(),
                sim_require_finite=True,
                sim_require_nnan=True,
                nc=nc,
            )
            return tuple(outs)

        devices = jax.devices()[:N_CORES]
        assert len(devices) == N_CORES
        self.mesh = Mesh(np.asarray(devices), ("core",))
        in_specs = (PartitionSpec("core"),) * (n_params + n_outs)
        out_specs = (PartitionSpec("core"),) * n_outs
        self.donate = tuple(range(n_params, n_params + n_outs))
        self.sharded = jax.jit(
            shard_map(
                _body,
                mesh=self.mesh,
                in_specs=in_specs,
                out_specs=out_specs,
                check_rep=False,
            ),
            donate_argnums=self.donate,
            keep_unused=True,
        )

    def stage_inputs(self, per_core_in):
        """per_core_in: list of dicts (len N_CORES) -> device-resident concat arrays."""
        import jax
        from jax.sharding import NamedSharding, PartitionSpec

        sh = NamedSharding(self.mesh, PartitionSpec("core"))
        staged = []
        for name in self.in_names:
            cat = np.concatenate(
                [np.asarray(per_core_in[c][name]) for c in range(N_CORES)], axis=0
            )
            staged.append(jax.device_put(cat, sh))
        return staged

    def fresh_outs(self):
        import jax
        from jax.sharding import NamedSharding, PartitionSpec

        sh = NamedSharding(self.mesh, PartitionSpec("core"))
        return [
            jax.device_put(
                np.zeros((N_CORES * z.shape[0], *z.shape[1:]), z.dtype), sh
            )
            for z in self.zero_outs
        ]

    def run(self, staged_in, out_bufs):
        import jax

        outs = self.sharded(*staged_in, *out_bufs)
        jax.block_until_ready(outs)
        return outs

    def results(self, outs):
        res = []
        for c in range(N_CORES):
            d = {}
            for i, name in enumerate(self.out_names):
                full = np.asarray(outs[i])
                d[name] = full.reshape(N_CORES, *self.out_avals[i].shape)[c]
            res.append(d)
        return res


def _get_runner(reps=1):
    if reps not in _RUNNERS:
        _RUNNERS[reps] = _Runner(reps)
    return _RUNNERS[reps]


# ---------------------------------------------------------------------------
# Host-side sharding / gather
def _per_core_inputs(x, w_qkv, w_out):
    x = np.asarray(x, dtype=np.float32)
    w_qkv = np.asarray(w_qkv, dtype=np.float32)
    w_out = np.asarray(w_out, dtype=np.float32)
    per_core = []
    for c in range(N_CORES):
        b, g = c // GROUPS, c % GROUPS
        hs = np.arange(g * HPG, (g + 1) * HPG)
        # qkv reshape order in reference: f = d*48 + k*16 + h
        rows_q = (np.arange(D)[None, :] * (3 * H_TOTAL) + hs[:, None]).reshape(-1)
        rows_k = rows_q + H_TOTAL
        rows_v = rows_q + 2 * H_TOTAL
        wqk = np.concatenate([w_qkv[rows_q], SCALE * w_qkv[rows_k]], axis=0)
        per_core.append(
            {
                "xT": np.ascontiguousarray(x[b].T).astype(np.float16),
                "wqkT": np.ascontiguousarray(wqk.T).astype(np.float16),
                "wvT": np.ascontiguousarray(w_qkv[rows_v].T).astype(np.float16),
                "woutT": np.ascontiguousarray(w_out[:, g * DV:(g + 1) * DV].T).astype(np.float16),
            }
        )
    return per_core


def kernel(x, w_qkv, w_out):
    runner = _get_runner(1)
    staged = runner.stage_inputs(_per_core_inputs(x, w_qkv, w_out))
    outs = runner.run(staged, runner.fresh_outs())
    res = runner.results(outs)
    y = np.zeros((B, T, E), dtype=np.float64)
    for c in range(N_CORES):
        y[c // GROUPS] += res[c]["y"].astype(np.float64)
    return y.astype(np.float32)
